# revision 25
# baseline (speedup 1.0000x reference)
"""Trainium2 Bass kernel for AdvancedGNNPredictorV2 (2-layer GAT + BN + mean-pool + MLP).

Contract: kernel(**inputs) takes FULL numpy inputs, returns FULL [512, 2] output.
Internally: nodes (and incident edges, partitioned by destination) are sharded
across 8 NeuronCores; small weights replicated; gather tables all-gathered; BN
stats and pooled graph sums all-reduced.

Self-contained: all shapes hardcoded for N=50000, E=800000, D_IN=128, H=4, C=64, G=512.
"""
import math
import numpy as np

from concourse import bass, bacc, mybir, tile
from concourse.bass_utils import run_bass_kernel_spmd

# Problem constants
N = 50000
E = 800000
D_IN = 128
H = 4
C = 64
HC = 256
G = 512
EPS = 1e-5
SLOPE = 0.2

M = 8                     # cores
NCN = N // M              # 6250 nodes per core
P = 128
NW = math.ceil(NCN / P)   # 49 node windows per core
LAST_ROWS = NCN - (NW - 1) * P  # 106
TW = HC + H               # 260 table row width (h || es)

F32 = mybir.dt.float32
BF16 = mybir.dt.bfloat16
F16 = mybir.dt.float16
I32 = mybir.dt.int32
I16 = mybir.dt.int16

# dtype knobs
TBL_DT = F16      # gather table dtype
STG_DT = F16      # staging (matmul rhs) dtype
IND_DT = F16      # indicator (matmul lhsT) dtype
TWP = 384         # padded table row elems (768B, 256B-multiple for dma_gather)
EDW = 128         # ed table row elems (256B)
WA = 25            # node windows in table half A
HALF_A = WA * P    # 3200 rows per core in half A
HALF_B = NCN - HALF_A  # 3050
A_ROWS = M * HALF_A    # 25600
B_ROWS = M * HALF_B    # 24400


# ---------------------------------------------------------------- host side

def _host_meta(edge_index: np.ndarray, batch: np.ndarray):
    """Partition/sort edges by destination; build per-core dma_gather metadata.

    The gather table is split into half A (each core's node windows 0..WA-1)
    and half B (the rest), all-gathered separately so AG overlaps the prelude.
    Each half has < 32768 rows, so int16 gather indices need no magnitude
    split. Per dst-window, edges are grouped into A-edges (src in half A) and
    B-edges, each padded to a chunk multiple (sentinels: row 0, drel=300).
    """
    src = edge_index[0].astype(np.int64)
    dst = edge_index[1].astype(np.int64)
    owner = dst // NCN

    # global node id -> (half, half-local row)
    s_c_all = src % NCN
    s_core = src // NCN
    in_a = s_c_all < HALF_A
    rowA = s_core * HALF_A + s_c_all
    rowB = s_core * HALF_B + (s_c_all - HALF_A)
    srow = np.where(in_a, rowA, rowB)

    percore = []
    na_max = nb_max = 1
    for c in range(M):
        m = owner == c
        sr, sa = srow[m], in_a[m]
        dl = (dst[m] - c * NCN).astype(np.int64)
        order = np.argsort(dl, kind="stable")
        sr, sa, dl = sr[order], sa[order], dl[order]
        w = dl // P
        aa, bb = [], []
        for wi in range(NW):
            sel = w == wi
            srw, saw, dlw = sr[sel], sa[sel], dl[sel]
            aa.append((srw[saw], dlw[saw]))
            bb.append((srw[~saw], dlw[~saw]))
            na_max = max(na_max, math.ceil(saw.sum() / P))
            nb_max = max(nb_max, math.ceil((~saw).sum() / P))
        percore.append((aa, bb))
    NLOW, NHIGH = int(na_max), int(nb_max)  # A-chunks, B-chunks per window
    K2 = NLOW + NHIGH

    def wrap16(a):
        return np.tile(a.reshape(-1, 16).T, (8, 1)).astype(np.int16)

    metas = []
    for c in range(M):
        aa, bb = percore[c]
        lowidx = np.zeros((P, NW * NLOW * 8), np.int16)
        highidx = np.zeros((P, NW * NHIGH * 8), np.int16)
        drel = np.full((P, NW * K2), 300.0, np.float16)
        drelT = np.full((NW, K2 * P), 300.0, np.float16)
        for wi in range(NW):
            sa_, da_ = aa[wi]
            sb_, db_ = bb[wi]
            na, nb = len(sa_), len(sb_)
            sap = np.zeros(NLOW * P, np.int64); sap[:na] = sa_
            sbp = np.zeros(NHIGH * P, np.int64); sbp[:nb] = sb_
            lowidx[:, wi * NLOW * 8 : (wi + 1) * NLOW * 8] = wrap16(sap)
            highidx[:, wi * NHIGH * 8 : (wi + 1) * NHIGH * 8] = wrap16(sbp)
            dr = np.full(K2 * P, 300.0, np.float32)
            dr[:na] = (da_ - wi * P).astype(np.float32)
            dr[NLOW * P : NLOW * P + nb] = (db_ - wi * P).astype(np.float32)
            drel[:, wi * K2 : (wi + 1) * K2] = dr.reshape(K2, P).T.astype(np.float16)
            drelT[wi, :] = dr.astype(np.float16)

        b_c = batch[c * NCN : (c + 1) * NCN].astype(np.int64)
        g_base = int(b_c[0])
        gr = (b_c - g_base).astype(np.float32)
        assert gr.max() < P, "more than 128 graphs per core not supported"
        gr_pad = np.full(NW * P, 500.0, np.float32)
        gr_pad[:NCN] = gr
        grel = gr_pad.reshape(NW, P).T.copy()
        scatidx = np.minimum(g_base + np.arange(P), G).astype(np.int32)[:, None]

        metas.append(
            dict(
                lowidx=np.ascontiguousarray(lowidx),
                highidx=np.ascontiguousarray(highidx),
                drel=np.ascontiguousarray(drel),
                drelT=np.ascontiguousarray(drelT),
                grel=np.ascontiguousarray(grel),
                scatidx=scatidx,
            )
        )
    return metas, (NLOW, NHIGH)


# ---------------------------------------------------------------- device program

def _build_program(split: tuple, dbg: bool = False):
    NLOW, NHIGH = split
    K_w = NLOW + NHIGH
    nc = bacc.Bacc("TRN2", target_bir_lowering=False, debug=False, num_devices=M, num_swdge_queues=2)
    RG = [list(range(M))]
    ECOLS = NW * K_w

    # I/O
    xc = nc.dram_tensor("xc", [NCN, D_IN], F16, kind="ExternalInput")
    wcat1 = nc.dram_tensor("wcat1", [D_IN, 2 * HC], F16, kind="ExternalInput")
    wcat2 = nc.dram_tensor("wcat2", [HC, 2 * HC], F16, kind="ExternalInput")
    asrc = [nc.dram_tensor(f"asrc{l}", [P, HC], F32, kind="ExternalInput") for l in (1, 2)]
    adst = [nc.dram_tensor(f"adst{l}", [P, HC], F32, kind="ExternalInput") for l in (1, 2)]
    biasr = [nc.dram_tensor(f"bias{l}", [P, HC], F32, kind="ExternalInput") for l in (1, 2)]
    bng = [nc.dram_tensor(f"bng{l}", [1, HC], F32, kind="ExternalInput") for l in (1, 2)]
    bnb = [nc.dram_tensor(f"bnb{l}", [1, HC], F32, kind="ExternalInput") for l in (1, 2)]
    fc1w = nc.dram_tensor("fc1w", [HC, 128], F32, kind="ExternalInput")
    fc2w = nc.dram_tensor("fc2w", [128, 2], F32, kind="ExternalInput")
    fc1b = nc.dram_tensor("fc1b", [P, 128], F32, kind="ExternalInput")
    fc2b = nc.dram_tensor("fc2b", [P, 2], F32, kind="ExternalInput")
    rcnt = nc.dram_tensor("rcnt", [G, 1], F32, kind="ExternalInput")
    grel_d = nc.dram_tensor("grel", [P, NW], F32, kind="ExternalInput")
    scatidx_d = nc.dram_tensor("scatidx", [P, 1], I32, kind="ExternalInput")
    lowidx_d = nc.dram_tensor("lowidx", [P, NW * NLOW * 8], I16, kind="ExternalInput")
    highidx_d = nc.dram_tensor("highidx", [P, NW * NHIGH * 8], I16, kind="ExternalInput")
    drel_d = nc.dram_tensor("drel", [P, ECOLS], F16, kind="ExternalInput")
    drelT_d = nc.dram_tensor("drelT", [NW, K_w * P], F16, kind="ExternalInput")
    out_d = nc.dram_tensor("out", [G, 2], F32, kind="ExternalOutput")
    if dbg:
        dbg_tfull = nc.dram_tensor("dbg_tfull", [A_ROWS, TWP], TBL_DT, kind="ExternalOutput")
        dbg_g = nc.dram_tensor("dbg_g", [P, K_w, TWP], F32, kind="ExternalOutput")
        dbg_stg = nc.dram_tensor("dbg_stg", [P, K_w, TW], F32, kind="ExternalOutput")
        dbg_ind = nc.dram_tensor("dbg_ind", [P, K_w, P], F32, kind="ExternalOutput")
        dbg_agg = nc.dram_tensor("dbg_agg", [P, TW], F32, kind="ExternalOutput")
        dbg_hpre1 = nc.dram_tensor("dbg_hpre1", [P, NW, HC], F32, kind="ExternalOutput")
        dbg_stats = nc.dram_tensor("dbg_stats", [1, 2 * HC], F32, kind="ExternalOutput")
        dbg_hbn1 = nc.dram_tensor("dbg_hbn1", [P, NW, HC], F32, kind="ExternalOutput")
        dbg_pool = nc.dram_tensor("dbg_pool", [G, HC], F32, kind="ExternalOutput")

    # Internal DRAM
    tshardA = [nc.dram_tensor(f"tshardA{l}", [HALF_A, TWP], TBL_DT, kind="Internal") for l in (1, 2)]
    tshardB = [nc.dram_tensor(f"tshardB{l}", [HALF_B, TWP], TBL_DT, kind="Internal") for l in (1, 2)]
    tfullA = [
        nc.dram_tensor(f"tfullA{l}", [A_ROWS, TWP], TBL_DT, kind="Internal", addr_space="Shared")
        for l in (1, 2)
    ]
    tfullB = [
        nc.dram_tensor(f"tfullB{l}", [B_ROWS, TWP], TBL_DT, kind="Internal", addr_space="Shared")
        for l in (1, 2)
    ]
    stats_in = [nc.dram_tensor(f"statsin{l}", [1, 2 * HC], F32, kind="Internal") for l in (1, 2)]
    stats_out = [
        nc.dram_tensor(f"statsout{l}", [1, 2 * HC], F32, kind="Internal", addr_space="Shared")
        for l in (1, 2)
    ]
    pooled_in = nc.dram_tensor("pooledin", [G + 1, HC], F32, kind="Internal")
    pooled_out = nc.dram_tensor("pooledout", [G, HC], F32, kind="Internal", addr_space="Shared")

    from concourse.masks import make_identity

    with tile.TileContext(nc) as tc:
        with (
            tc.tile_pool(name="persist", bufs=1) as pp,
            tc.tile_pool(name="const", bufs=1) as cp,
            tc.tile_pool(name="work", bufs=2) as wp,
            tc.tile_pool(name="edge", bufs=2) as ep,
            tc.tile_pool(name="gath", bufs=2) as gp,
            tc.tile_pool(name="gbig", bufs=3) as gbp,
            tc.tile_pool(name="psum_tr", bufs=2, space="PSUM") as ptr,
            tc.tile_pool(name="psum_mm", bufs=2, space="PSUM") as pmm,
            tc.tile_pool(name="psum_agg", bufs=2, space="PSUM") as pagg,
            tc.tile_pool(name="psum_st", bufs=2, space="PSUM") as pst,
            # bank budget: tr 2 + psum_h 2 + agg 2 + st 2 = 8
        ):
            # ---------------- constants
            identity = cp.tile([P, P], F32)
            make_identity(nc, identity[:])
            identity_h = cp.tile([P, P], F16)
            nc.vector.tensor_copy(identity_h[:], identity[:])
            iota_i = cp.tile([P, P], I32)
            nc.gpsimd.iota(iota_i[:], pattern=[[1, P]], base=0, channel_multiplier=0)
            iota_f = cp.tile([P, P], F32)
            nc.vector.tensor_copy(iota_f[:], iota_i[:])
            iota_h = cp.tile([P, P], F16)
            nc.vector.tensor_copy(iota_h[:], iota_i[:])
            iota_ci = cp.tile([P, 1], I32)
            nc.gpsimd.iota(iota_ci[:], pattern=[[0, 1]], base=0, channel_multiplier=1)
            iota_col = cp.tile([P, 1], F32)
            nc.vector.tensor_copy(iota_col[:], iota_ci[:])
            ones_col = cp.tile([P, 1], F16)
            nc.gpsimd.memset(ones_col[:], 1.0)

            wcat1_s = cp.tile([D_IN, 2 * HC], F16)
            nc.sync.dma_start(wcat1_s[:], wcat1[:])
            wcat2_s0 = cp.tile([P, 2 * HC], F16)
            nc.sync.dma_start(wcat2_s0[:], wcat2[0:P, :])
            wcat2_s1 = cp.tile([P, 2 * HC], F16)
            nc.sync.dma_start(wcat2_s1[:], wcat2[P:HC, :])
            asrc_s = [cp.tile([P, HC], F32, name=f"asrc_s{l}") for l in range(2)]
            adst_s = [cp.tile([P, HC], F32, name=f"adst_s{l}") for l in range(2)]
            bias_s = [cp.tile([P, HC], F32, name=f"bias_s{l}") for l in range(2)]
            for l in range(2):
                nc.sync.dma_start(asrc_s[l][:], asrc[l][:])
                nc.sync.dma_start(adst_s[l][:], adst[l][:])
                nc.sync.dma_start(bias_s[l][:], biasr[l][:])

            # persistent node buffers
            h_pre = pp.tile([P, NW, HC], F16)   # pre/post BN node features
            ed_sb = pp.tile([P, NW, H], F32)    # per-node a_dst scores (this layer)
            nc.gpsimd.memset(h_pre[:], 0.0)

            def rows_of(w):
                return LAST_ROWS if w == NW - 1 else P

            # ================= per-layer =================
            for l in range(2):
                # ---- prelude: GEMM h||res, es/ed, build tables
                stats_acc = wp.tile([1, 2 * HC], F32, tag="stats_acc")
                nc.gpsimd.memset(stats_acc[:], 0.0)

                for w in range(NW):
                    rows = rows_of(w)
                    psum_h = pmm.tile([P, 2 * HC], F32, tag="psum_h")
                    if l == 0:
                        x_t = wp.tile([P, D_IN], F16, tag="x_t")
                        nc.sync.dma_start(x_t[:rows], xc[w * P : w * P + rows, :])
                        tr = ptr.tile([P, P], F16, tag="tr")
                        nc.tensor.transpose(tr[:, :rows], x_t[:rows, :], identity_h[:rows, :rows])
                        xT = wp.tile([P, P], F16, tag="xT")
                        nc.scalar.copy(xT[:, :rows], tr[:, :rows])
                        nc.tensor.matmul(
                            psum_h[:rows], lhsT=xT[:, :rows], rhs=wcat1_s[:], start=True, stop=True
                        )
                    else:
                        hin = h_pre[:, w, :]
                        for k in range(2):
                            tr = ptr.tile([P, P], F16, tag="tr")
                            nc.tensor.transpose(
                                tr[:, :rows], hin[:rows, k * P : (k + 1) * P], identity_h[:rows, :rows]
                            )
                            hT = wp.tile([P, P], F16, tag="xT")
                            nc.scalar.copy(hT[:, :rows], tr[:, :rows])
                            nc.tensor.matmul(
                                psum_h[:rows],
                                lhsT=hT[:, :rows],
                                rhs=(wcat2_s0 if k == 0 else wcat2_s1)[:],
                                start=(k == 0),
                                stop=(k == 1),
                            )
                    h_t = wp.tile([P, HC], F32, tag="h_t")
                    nc.scalar.copy(h_t[:rows], psum_h[:rows, 0:HC])
                    # residual + bias -> h_pre tile (overwrites layer-(l-1) values,
                    # which only this window's GEMM reads)
                    nc.vector.tensor_add(
                        h_pre[:rows, w, :], psum_h[:rows, HC : 2 * HC], bias_s[l][:rows]
                    )
                    # es / ed
                    scr = wp.tile([P, HC], F32, tag="scr")
                    es_t = wp.tile([P, H], F32, tag="es_t")
                    nc.vector.tensor_tensor(
                        out=scr[:rows], in0=h_t[:rows], in1=asrc_s[l][:rows], op=mybir.AluOpType.mult
                    )
                    nc.vector.reduce_sum(
                        out=es_t[:rows],
                        in_=scr[:rows].rearrange("p (h c) -> p h c", h=H),
                        axis=mybir.AxisListType.X,
                    )
                    nc.vector.tensor_tensor(
                        out=scr[:rows], in0=h_t[:rows], in1=adst_s[l][:rows], op=mybir.AluOpType.mult
                    )
                    nc.vector.reduce_sum(
                        out=ed_sb[:rows, w, :],
                        in_=scr[:rows].rearrange("p (h c) -> p h c", h=H),
                        axis=mybir.AxisListType.X,
                    )
                    # write table shard + ed table (fp16)
                    h_tb = wp.tile([P, HC], TBL_DT, tag="h_tb")
                    nc.scalar.copy(h_tb[:rows], h_t[:rows])
                    es_tb = wp.tile([P, H], TBL_DT, tag="es_tb")
                    nc.scalar.copy(es_tb[:rows], es_t[:rows])
                    if w < WA:
                        ro = w * P
                        nc.sync.dma_start(tshardA[l][ro : ro + rows, 0:HC], h_tb[:rows])
                        nc.sync.dma_start(tshardA[l][ro : ro + rows, HC:TW], es_tb[:rows])
                    else:
                        ro = (w - WA) * P
                        nc.sync.dma_start(tshardB[l][ro : ro + rows, 0:HC], h_tb[:rows])
                        nc.sync.dma_start(tshardB[l][ro : ro + rows, HC:TW], es_tb[:rows])
                    if w == WA - 1:
                        nc.gpsimd.collective_compute(
                            "AllGather",
                            mybir.AluOpType.bypass,
                            replica_groups=RG,
                            ins=[tshardA[l].ap()],
                            outs=[tfullA[l].ap()],
                        )

                # ---- all-gather table half B (half A launched mid-prelude)
                nc.gpsimd.collective_compute(
                    "AllGather",
                    mybir.AluOpType.bypass,
                    replica_groups=RG,
                    ins=[tshardB[l].ap()],
                    outs=[tfullB[l].ap()],
                )

                if dbg and l == 0:
                    nc.sync.dma_start(dbg_tfull[:], tfullA[0][:])

                # ---- edge pass; A-gathers prefetched PF windows ahead so they
                # run during AllGather-B and ahead of per-window compute
                PF = 2
                gtiles = {}

                def emit_gather_a(w):
                    li = gp.tile([P, NLOW * 8], I16, tag="li")
                    nc.sync.dma_start(li[:], lowidx_d[:, w * NLOW * 8 : (w + 1) * NLOW * 8])
                    g = gbp.tile([P, K_w, TWP], TBL_DT, tag="g")
                    nc.gpsimd.dma_gather(
                        out_ap=g[:, 0:NLOW, :],
                        in_ap=tfullA[l][:],
                        idxs_ap=li[:],
                        num_idxs=NLOW * P,
                        num_idxs_reg=NLOW * P,
                        elem_size=TWP,
                        single_packet=False,
                        queue_num=0,
                    )
                    gtiles[w] = g

                for w in range(PF):
                    emit_gather_a(w)

                for w in range(NW):
                    rows = rows_of(w)
                    g = gtiles.pop(w)
                    hi = gp.tile([P, NHIGH * 8], I16, tag="hi")
                    nc.sync.dma_start(hi[:], highidx_d[:, w * NHIGH * 8 : (w + 1) * NHIGH * 8])
                    drel = gp.tile([P, K_w], F16, tag="drel")
                    nc.sync.dma_start(drel[:], drel_d[:, w * K_w : (w + 1) * K_w])
                    drelT_rep = gp.tile([P, K_w * P], F16, tag="drelT_rep")
                    nc.sync.dma_start(
                        drelT_rep[:], drelT_d[w : w + 1, :].to_broadcast([P, K_w * P])
                    )
                    nc.gpsimd.dma_gather(
                        out_ap=g[:, NLOW:K_w, :],
                        in_ap=tfullB[l][:],
                        idxs_ap=hi[:],
                        num_idxs=NHIGH * P,
                        num_idxs_reg=NHIGH * P,
                        elem_size=TWP,
                        single_packet=False,
                        queue_num=1,
                    )
                    if w + PF < NW:
                        emit_gather_a(w + PF)
                    # ed broadcast node->edge via transposed indicator matmuls
                    ind_T = ep.tile([P, K_w, P], F16, tag="indT")
                    nc.vector.tensor_scalar(
                        out=ind_T[:],
                        in0=drelT_rep[:].rearrange("d (t e) -> d t e", e=P),
                        scalar1=iota_col[:, 0:1],
                        scalar2=None,
                        op0=mybir.AluOpType.is_equal,
                    )
                    ed16 = wp.tile([P, H], F16, tag="ed16")
                    nc.scalar.copy(ed16[:], ed_sb[:, w, :])
                    psum_ed = ptr.tile([P, K_w * H], F32, tag="tr")
                    for t in range(K_w):
                        nc.tensor.matmul(
                            psum_ed[:, t * H : (t + 1) * H],
                            lhsT=ind_T[:, t, :],
                            rhs=ed16[:],
                            start=True,
                            stop=True,
                        )
                    edv = wp.tile([P, K_w * H], F32, tag="edv")
                    nc.scalar.copy(edv[:], psum_ed[:])
                    # ex = exp(lrelu(es + ed))
                    s_t = wp.tile([P, K_w, H], F32, tag="s_t")
                    nc.vector.tensor_tensor(
                        out=s_t[:],
                        in0=g[:, :, HC:TW],
                        in1=edv[:].rearrange("p (t h) -> p t h", h=H),
                        op=mybir.AluOpType.add,
                    )
                    lr_t = wp.tile([P, K_w, H], F32, tag="lr_t")
                    nc.vector.tensor_scalar_mul(lr_t[:], s_t[:], SLOPE)
                    nc.vector.tensor_tensor(
                        out=lr_t[:], in0=lr_t[:], in1=s_t[:], op=mybir.AluOpType.max
                    )
                    ex_t = wp.tile([P, K_w, H], F16, tag="ex_t")
                    nc.scalar.activation(ex_t[:], lr_t[:], mybir.ActivationFunctionType.Exp)
                    # indicator
                    ind = ep.tile([P, K_w, P], IND_DT, tag="ind")
                    nc.vector.tensor_tensor(
                        out=ind[:],
                        in0=drel[:, :, None].to_broadcast([P, K_w, P]),
                        in1=iota_h[:, None, :].to_broadcast([P, K_w, P]),
                        op=mybir.AluOpType.is_equal,
                    )
                    # staging: [g*ex || ex]
                    stg = ep.tile([P, K_w, TW], STG_DT, tag="stg")
                    nc.vector.tensor_tensor(
                        out=stg[:, :, 0:HC].rearrange("p t (h c) -> p t h c", h=H),
                        in0=g[:, :, 0:HC].rearrange("p t (h c) -> p t h c", h=H),
                        in1=ex_t[:, :, :, None].to_broadcast([P, K_w, H, C]),
                        op=mybir.AluOpType.mult,
                    )
                    nc.scalar.copy(stg[:, :, HC:TW], ex_t[:])
                    if dbg and l == 0 and w == 0:
                        nc.gpsimd.dma_start(dbg_g[:], g[:])
                        nc.gpsimd.dma_start(dbg_stg[:], stg[:])
                        nc.gpsimd.dma_start(dbg_ind[:], ind[:])
                    # aggregate
                    agg = pagg.tile([P, TW], F32, tag="agg")
                    for t in range(K_w):
                        nc.tensor.matmul(
                            agg[:],
                            lhsT=ind[:, t, :],
                            rhs=stg[:, t, :],
                            start=(t == 0),
                            stop=(t == K_w - 1),
                        )
                    if dbg and l == 0 and w == 0:
                        aggc = wp.tile([P, TW], F32, tag="aggc")
                        nc.scalar.copy(aggc[:], agg[:])
                        nc.sync.dma_start(dbg_agg[:], aggc[:])
                    # evict: h_pre = agg[:, :256] * rden + res_b
                    den = wp.tile([P, H], F32, tag="den")
                    nc.vector.tensor_scalar_add(den[:rows], agg[:rows, HC:TW], 1e-16)
                    rden = wp.tile([P, H], F32, tag="rden")
                    nc.vector.reciprocal(rden[:rows], den[:rows])
                    agn = wp.tile([P, HC], F32, tag="agn")
                    nc.vector.tensor_tensor(
                        out=agn[:rows].rearrange("p (h c) -> p h c", h=H),
                        in0=agg[:rows, 0:HC].rearrange("p (h c) -> p h c", h=H),
                        in1=rden[:rows, :, None].to_broadcast([rows, H, C]),
                        op=mybir.AluOpType.mult,
                    )
                    nc.vector.tensor_add(h_pre[:rows, w, :], agn[:rows], h_pre[:rows, w, :])
                    # BN stats
                    hsq = wp.tile([P, HC], F16, tag="hsq")
                    nc.scalar.square(hsq[:rows], h_pre[:rows, w, :])
                    st = pst.tile([1, 2 * HC], F32, tag="st")
                    nc.tensor.matmul(
                        st[0:1, 0:HC], lhsT=ones_col[:rows, :], rhs=h_pre[:rows, w, :], start=True, stop=True
                    )
                    nc.tensor.matmul(
                        st[0:1, HC : 2 * HC], lhsT=ones_col[:rows, :], rhs=hsq[:rows], start=True, stop=True
                    )
                    nc.vector.tensor_add(stats_acc[:], stats_acc[:], st[:])

                if dbg and l == 0:
                    nc.gpsimd.dma_start(dbg_hpre1[:], h_pre[:])
                # ---- BN all-reduce + apply + relu
                nc.sync.dma_start(stats_in[l][:], stats_acc[:])
                nc.gpsimd.collective_compute(
                    "AllReduce",
                    mybir.AluOpType.add,
                    replica_groups=RG,
                    ins=[stats_in[l].ap()],
                    outs=[stats_out[l].ap()],
                )
                stats_g = wp.tile([1, 2 * HC], F32, tag="stats_g")
                nc.sync.dma_start(stats_g[:], stats_out[l][:])
                bngs = wp.tile([1, HC], F32, tag="bngs")
                nc.sync.dma_start(bngs[:], bng[l][:])
                bnbs = wp.tile([1, HC], F32, tag="bnbs")
                nc.sync.dma_start(bnbs[:], bnb[l][:])
                mrow = wp.tile([1, HC], F32, tag="mrow")
                nc.vector.tensor_scalar_mul(mrow[:], stats_g[0:1, 0:HC], 1.0 / N)
                vrow = wp.tile([1, HC], F32, tag="vrow")
                nc.vector.tensor_scalar_mul(vrow[:], stats_g[0:1, HC : 2 * HC], 1.0 / N)
                msq = wp.tile([1, HC], F32, tag="msq")
                nc.vector.tensor_tensor(out=msq[:], in0=mrow[:], in1=mrow[:], op=mybir.AluOpType.mult)
                nc.vector.tensor_tensor(out=vrow[:], in0=vrow[:], in1=msq[:], op=mybir.AluOpType.subtract)
                nc.vector.tensor_scalar_add(vrow[:], vrow[:], EPS)
                sq = wp.tile([1, HC], F32, tag="sq")
                nc.scalar.activation(sq[:], vrow[:], mybir.ActivationFunctionType.Sqrt)
                inv = wp.tile([1, HC], F32, tag="inv")
                nc.vector.reciprocal(inv[:], sq[:])
                scale = wp.tile([1, HC], F32, tag="scale")
                nc.vector.tensor_tensor(out=scale[:], in0=inv[:], in1=bngs[:], op=mybir.AluOpType.mult)
                shift = wp.tile([1, HC], F32, tag="shift")
                nc.vector.tensor_tensor(out=shift[:], in0=mrow[:], in1=scale[:], op=mybir.AluOpType.mult)
                nc.vector.tensor_tensor(out=shift[:], in0=bnbs[:], in1=shift[:], op=mybir.AluOpType.subtract)
                scale_rep = wp.tile([P, HC], F32, tag="scale_rep")
                nc.gpsimd.partition_broadcast(scale_rep[:], scale[:])
                shift_rep = wp.tile([P, HC], F32, tag="shift_rep")
                nc.gpsimd.partition_broadcast(shift_rep[:], shift[:])
                for w in range(NW):
                    rows = rows_of(w)
                    nc.vector.tensor_tensor(
                        out=h_pre[:rows, w, :], in0=h_pre[:rows, w, :], in1=scale_rep[:rows],
                        op=mybir.AluOpType.mult,
                    )
                    nc.vector.tensor_tensor(
                        out=h_pre[:rows, w, :], in0=h_pre[:rows, w, :], in1=shift_rep[:rows],
                        op=mybir.AluOpType.add,
                    )
                    nc.scalar.activation(
                        h_pre[:rows, w, :], h_pre[:rows, w, :], mybir.ActivationFunctionType.Relu
                    )
                if dbg and l == 0:
                    nc.sync.dma_start(dbg_stats[:], stats_out[0][:])
                    nc.gpsimd.dma_start(dbg_hbn1[:], h_pre[:])

            # ================= pooling =================
            pool_ps = pmm.tile([P, HC], F32, tag="psum_h")
            for w in range(NW):
                rows = rows_of(w)
                grt = wp.tile([P, 1], F32, tag="grt")
                nc.sync.dma_start(grt[:], grel_d[:, w : w + 1])
                gind = wp.tile([P, P], F16, tag="gind")
                nc.vector.tensor_tensor(
                    out=gind[:],
                    in0=grt[:].to_broadcast([P, P]),
                    in1=iota_f[:],
                    op=mybir.AluOpType.is_equal,
                )
                nc.tensor.matmul(
                    pool_ps[:],
                    lhsT=gind[:rows, :],
                    rhs=h_pre[:rows, w, :],
                    start=(w == 0),
                    stop=(w == NW - 1),
                )
            pooled_sb = wp.tile([P, HC], F32, tag="pooled_sb")
            nc.scalar.copy(pooled_sb[:], pool_ps[:])
            zt = wp.tile([P, HC], F32, tag="zt")
            nc.gpsimd.memset(zt[:], 0.0)
            for r in range(0, G, P):
                nc.sync.dma_start(pooled_in[r : r + P, :], zt[:])
            nc.sync.dma_start(pooled_in[G : G + 1, :], zt[0:1, :])
            scat = wp.tile([P, 1], I32, tag="scat")
            nc.sync.dma_start(scat[:], scatidx_d[:])
            nc.gpsimd.indirect_dma_start(
                out=pooled_in[:],
                out_offset=bass.IndirectOffsetOnAxis(ap=scat[:, :1], axis=0),
                in_=pooled_sb[:],
                in_offset=None,
            )
            nc.gpsimd.collective_compute(
                "AllReduce",
                mybir.AluOpType.add,
                replica_groups=RG,
                ins=[pooled_in[0:G, :]],
                outs=[pooled_out.ap()],
            )

            if dbg:
                nc.sync.dma_start(dbg_pool[:], pooled_out[:])

            # ================= MLP =================
            fc1w_s0 = cp.tile([P, 128], F32)
            nc.sync.dma_start(fc1w_s0[:], fc1w[0:P, :])
            fc1w_s1 = cp.tile([P, 128], F32)
            nc.sync.dma_start(fc1w_s1[:], fc1w[P:HC, :])
            fc2w_s = cp.tile([P, 2], F32)
            nc.sync.dma_start(fc2w_s[:], fc2w[:])
            fc1b_s = cp.tile([P, 128], F32)
            nc.sync.dma_start(fc1b_s[:], fc1b[:])
            fc2b_s = cp.tile([P, 2], F32)
            nc.sync.dma_start(fc2b_s[:], fc2b[:])

            for tgi in range(G // P):
                pt = wp.tile([P, HC], F32, tag="pt")
                nc.sync.dma_start(pt[:], pooled_out[tgi * P : (tgi + 1) * P, :])
                rc = wp.tile([P, 1], F32, tag="rc")
                nc.sync.dma_start(rc[:], rcnt[tgi * P : (tgi + 1) * P, :])
                nc.vector.tensor_scalar_mul(pt[:], pt[:], rc[:, 0:1])
                mm1 = pmm.tile([P, 128], F32, tag="psum_h")
                for k in range(2):
                    tr = ptr.tile([P, P], F32, tag="tr")
                    nc.tensor.transpose(tr[:], pt[:, k * P : (k + 1) * P], identity[:])
                    ptT = wp.tile([P, P], F32, tag="ptT")
                    nc.scalar.copy(ptT[:], tr[:])
                    nc.tensor.matmul(
                        mm1[:],
                        lhsT=ptT[:],
                        rhs=(fc1w_s0 if k == 0 else fc1w_s1)[:],
                        start=(k == 0),
                        stop=(k == 1),
                    )
                o1 = wp.tile([P, 128], F32, tag="o1")
                nc.vector.tensor_add(o1[:], mm1[:], fc1b_s[:])
                nc.scalar.activation(o1[:], o1[:], mybir.ActivationFunctionType.Relu)
                tr2 = ptr.tile([P, P], F32, tag="tr")
                nc.tensor.transpose(tr2[:], o1[:], identity[:])
                o1T = wp.tile([P, P], F32, tag="o1T")
                nc.scalar.copy(o1T[:], tr2[:])
                mm2 = pagg.tile([P, 2], F32, tag="agg")
                nc.tensor.matmul(mm2[:], lhsT=o1T[:], rhs=fc2w_s[:], start=True, stop=True)
                oo = wp.tile([P, 2], F32, tag="oo")
                nc.vector.tensor_add(oo[:], mm2[:], fc2b_s[:])
                nc.sync.dma_start(out_d[tgi * P : (tgi + 1) * P, :], oo[:])

    nc.compile()
    return nc


_PROGRAM_CACHE: dict[tuple, object] = {}


def _get_program(split: tuple):
    if split not in _PROGRAM_CACHE:
        _PROGRAM_CACHE[split] = _build_program(split)
    return _PROGRAM_CACHE[split]


# ---------------------------------------------------------------- entry point

def kernel(
    x, edge_index, edge_attr, batch,
    W1, att_src1, att_dst1, b1, res1_W, res1_b, bn1_g, bn1_b,
    W2, att_src2, att_dst2, b2, res2_W, res2_b, bn2_g, bn2_b,
    fc1_W, fc1_b, fc2_W, fc2_b,
    _run_opts: dict | None = None,
):
    x = np.asarray(x, np.float32)
    edge_index = np.asarray(edge_index)
    batch = np.asarray(batch)

    metas, split = _host_meta(edge_index, batch)
    nc = _get_program(split)

    wcat1 = np.ascontiguousarray(np.concatenate([np.asarray(W1), np.asarray(res1_W)], axis=1)).astype(np.float16)
    wcat2 = np.ascontiguousarray(np.concatenate([np.asarray(W2), np.asarray(res2_W)], axis=1)).astype(np.float16)
    rep = lambda v: np.ascontiguousarray(np.tile(np.asarray(v, np.float32).reshape(1, -1), (P, 1)))
    cnt = np.bincount(batch, minlength=G).astype(np.float32)
    rcnt = (1.0 / np.maximum(cnt, 1.0)).astype(np.float32)[:, None]

    shared = dict(
        wcat1=wcat1, wcat2=wcat2,
        asrc1=rep(np.asarray(att_src1).reshape(-1)), adst1=rep(np.asarray(att_dst1).reshape(-1)),
        asrc2=rep(np.asarray(att_src2).reshape(-1)), adst2=rep(np.asarray(att_dst2).reshape(-1)),
        bias1=rep(np.asarray(b1) + np.asarray(res1_b)), bias2=rep(np.asarray(b2) + np.asarray(res2_b)),
        bng1=np.asarray(bn1_g, np.float32).reshape(1, -1), bnb1=np.asarray(bn1_b, np.float32).reshape(1, -1),
        bng2=np.asarray(bn2_g, np.float32).reshape(1, -1), bnb2=np.asarray(bn2_b, np.float32).reshape(1, -1),
        fc1w=np.asarray(fc1_W, np.float32), fc2w=np.asarray(fc2_W, np.float32),
        fc1b=rep(fc1_b), fc2b=rep(fc2_b),
        rcnt=rcnt,
    )
    in_maps = []
    for c in range(M):
        m = metas[c]
        in_maps.append(
            dict(
                shared,
                xc=np.ascontiguousarray(x[c * NCN : (c + 1) * NCN]).astype(np.float16),
                grel=m["grel"], scatidx=m["scatidx"],
                lowidx=m["lowidx"], highidx=m["highidx"], drel=m["drel"], drelT=m["drelT"],
            )
        )

    opts = _run_opts or {}
    res = run_bass_kernel_spmd(nc, in_maps, core_ids=list(range(M)), **opts)
    out = res.results[0]["out"].astype(np.float32)
    if opts:
        kernel.last_result = res  # stash for profiling harnesses
    return out


# revision 26
# speedup vs baseline: 1.0985x; 1.0985x over previous
"""Trainium2 Bass kernel for AdvancedGNNPredictorV2 (2-layer GAT + BN + mean-pool + MLP).

Contract: kernel(**inputs) takes FULL numpy inputs, returns FULL [512, 2] output.
Internally: nodes (and incident edges, partitioned by destination) are sharded
across 8 NeuronCores; small weights replicated; gather tables all-gathered; BN
stats and pooled graph sums all-reduced.

Self-contained: all shapes hardcoded for N=50000, E=800000, D_IN=128, H=4, C=64, G=512.
"""
import math
import numpy as np

from concourse import bass, bacc, mybir, tile
from concourse.bass_utils import run_bass_kernel_spmd

# Problem constants
N = 50000
E = 800000
D_IN = 128
H = 4
C = 64
HC = 256
G = 512
EPS = 1e-5
SLOPE = 0.2

M = 8                     # cores
NCN = N // M              # 6250 nodes per core
P = 128
NW = math.ceil(NCN / P)   # 49 node windows per core
LAST_ROWS = NCN - (NW - 1) * P  # 106
TW = HC + H               # 260 table row width (h || es)

F32 = mybir.dt.float32
BF16 = mybir.dt.bfloat16
F16 = mybir.dt.float16
I32 = mybir.dt.int32
I16 = mybir.dt.int16

# dtype knobs
TBL_DT = F16      # gather table dtype
STG_DT = F16      # staging (matmul rhs) dtype
IND_DT = F16      # indicator (matmul lhsT) dtype
TWP = 384         # padded table row elems (768B, 256B-multiple for dma_gather)
EDW = 128         # ed table row elems (256B)
WA = 25            # node windows in table half A
HALF_A = WA * P    # 3200 rows per core in half A
HALF_B = NCN - HALF_A  # 3050
A_ROWS = M * HALF_A    # 25600
B_ROWS = M * HALF_B    # 24400


# ---------------------------------------------------------------- host side

def _host_meta(edge_index: np.ndarray, batch: np.ndarray):
    """Partition/sort edges by destination; build per-core dma_gather metadata.

    The gather table is split into half A (each core's node windows 0..WA-1)
    and half B (the rest), all-gathered separately so AG overlaps the prelude.
    Each half has < 32768 rows, so int16 gather indices need no magnitude
    split. Per dst-window, edges are grouped into A-edges (src in half A) and
    B-edges, each padded to a chunk multiple (sentinels: row 0, drel=300).
    """
    src = edge_index[0].astype(np.int64)
    dst = edge_index[1].astype(np.int64)
    owner = dst // NCN

    # global node id -> (half, half-local row)
    s_c_all = src % NCN
    s_core = src // NCN
    in_a = s_c_all < HALF_A
    rowA = s_core * HALF_A + s_c_all
    rowB = s_core * HALF_B + (s_c_all - HALF_A)
    srow = np.where(in_a, rowA, rowB)

    percore = []
    na_max = nb_max = 1
    for c in range(M):
        m = owner == c
        sr, sa = srow[m], in_a[m]
        dl = (dst[m] - c * NCN).astype(np.int64)
        order = np.argsort(dl, kind="stable")
        sr, sa, dl = sr[order], sa[order], dl[order]
        w = dl // P
        aa, bb = [], []
        for wi in range(NW):
            sel = w == wi
            srw, saw, dlw = sr[sel], sa[sel], dl[sel]
            aa.append((srw[saw], dlw[saw]))
            bb.append((srw[~saw], dlw[~saw]))
            na_max = max(na_max, math.ceil(saw.sum() / P))
            nb_max = max(nb_max, math.ceil((~saw).sum() / P))
        percore.append((aa, bb))
    NLOW, NHIGH = int(na_max), int(nb_max)  # A-chunks, B-chunks per window
    K2 = NLOW + NHIGH

    def wrap16(a):
        return np.tile(a.reshape(-1, 16).T, (8, 1)).astype(np.int16)

    metas = []
    for c in range(M):
        aa, bb = percore[c]
        lowidx = np.zeros((P, NW * NLOW * 8), np.int16)
        highidx = np.zeros((P, NW * NHIGH * 8), np.int16)
        drel = np.full((P, NW * K2), 300.0, np.float16)
        drelT = np.full((NW, K2 * P), 300.0, np.float16)
        for wi in range(NW):
            sa_, da_ = aa[wi]
            sb_, db_ = bb[wi]
            na, nb = len(sa_), len(sb_)
            sap = np.zeros(NLOW * P, np.int64); sap[:na] = sa_
            sbp = np.zeros(NHIGH * P, np.int64); sbp[:nb] = sb_
            lowidx[:, wi * NLOW * 8 : (wi + 1) * NLOW * 8] = wrap16(sap)
            highidx[:, wi * NHIGH * 8 : (wi + 1) * NHIGH * 8] = wrap16(sbp)
            dr = np.full(K2 * P, 300.0, np.float32)
            dr[:na] = (da_ - wi * P).astype(np.float32)
            dr[NLOW * P : NLOW * P + nb] = (db_ - wi * P).astype(np.float32)
            drel[:, wi * K2 : (wi + 1) * K2] = dr.reshape(K2, P).T.astype(np.float16)
            drelT[wi, :] = dr.astype(np.float16)

        b_c = batch[c * NCN : (c + 1) * NCN].astype(np.int64)
        g_base = int(b_c[0])
        gr = (b_c - g_base).astype(np.float32)
        assert gr.max() < P, "more than 128 graphs per core not supported"
        gr_pad = np.full(NW * P, 500.0, np.float32)
        gr_pad[:NCN] = gr
        grel = gr_pad.reshape(NW, P).T.copy()
        scatidx = np.minimum(g_base + np.arange(P), G).astype(np.int32)[:, None]

        metas.append(
            dict(
                lowidx=np.ascontiguousarray(lowidx),
                highidx=np.ascontiguousarray(highidx),
                drel=np.ascontiguousarray(drel),
                drelT=np.ascontiguousarray(drelT),
                grel=np.ascontiguousarray(grel),
                scatidx=scatidx,
            )
        )
    return metas, (NLOW, NHIGH)


# ---------------------------------------------------------------- device program

def _build_program(split: tuple, dbg: bool = False):
    NLOW, NHIGH = split
    K_w = NLOW + NHIGH
    nc = bacc.Bacc("TRN2", target_bir_lowering=False, debug=False, num_devices=M, num_swdge_queues=2)
    RG = [list(range(M))]
    ECOLS = NW * K_w

    # I/O
    xc = nc.dram_tensor("xc", [NCN, D_IN], F16, kind="ExternalInput")
    wcat1 = nc.dram_tensor("wcat1", [D_IN, 2 * HC], F16, kind="ExternalInput")
    wcat2 = nc.dram_tensor("wcat2", [HC, 2 * HC], F16, kind="ExternalInput")
    asrc = [nc.dram_tensor(f"asrc{l}", [P, HC], F32, kind="ExternalInput") for l in (1, 2)]
    adst = [nc.dram_tensor(f"adst{l}", [P, HC], F32, kind="ExternalInput") for l in (1, 2)]
    biasr = [nc.dram_tensor(f"bias{l}", [P, HC], F32, kind="ExternalInput") for l in (1, 2)]
    bng = [nc.dram_tensor(f"bng{l}", [1, HC], F32, kind="ExternalInput") for l in (1, 2)]
    bnb = [nc.dram_tensor(f"bnb{l}", [1, HC], F32, kind="ExternalInput") for l in (1, 2)]
    fc1w = nc.dram_tensor("fc1w", [HC, 128], F32, kind="ExternalInput")
    fc2w = nc.dram_tensor("fc2w", [128, 2], F32, kind="ExternalInput")
    fc1b = nc.dram_tensor("fc1b", [P, 128], F32, kind="ExternalInput")
    fc2b = nc.dram_tensor("fc2b", [P, 2], F32, kind="ExternalInput")
    rcnt = nc.dram_tensor("rcnt", [G, 1], F32, kind="ExternalInput")
    grel_d = nc.dram_tensor("grel", [P, NW], F32, kind="ExternalInput")
    scatidx_d = nc.dram_tensor("scatidx", [P, 1], I32, kind="ExternalInput")
    lowidx_d = nc.dram_tensor("lowidx", [P, NW * NLOW * 8], I16, kind="ExternalInput")
    highidx_d = nc.dram_tensor("highidx", [P, NW * NHIGH * 8], I16, kind="ExternalInput")
    drel_d = nc.dram_tensor("drel", [P, ECOLS], F16, kind="ExternalInput")
    drelT_d = nc.dram_tensor("drelT", [NW, K_w * P], F16, kind="ExternalInput")
    out_d = nc.dram_tensor("out", [G, 2], F32, kind="ExternalOutput")
    if dbg:
        dbg_tfull = nc.dram_tensor("dbg_tfull", [A_ROWS, TWP], TBL_DT, kind="ExternalOutput")
        dbg_g = nc.dram_tensor("dbg_g", [P, K_w, TWP], F32, kind="ExternalOutput")
        dbg_stg = nc.dram_tensor("dbg_stg", [P, K_w, TW], F32, kind="ExternalOutput")
        dbg_ind = nc.dram_tensor("dbg_ind", [P, K_w, P], F32, kind="ExternalOutput")
        dbg_agg = nc.dram_tensor("dbg_agg", [P, TW], F32, kind="ExternalOutput")
        dbg_hpre1 = nc.dram_tensor("dbg_hpre1", [P, NW, HC], F32, kind="ExternalOutput")
        dbg_stats = nc.dram_tensor("dbg_stats", [1, 2 * HC], F32, kind="ExternalOutput")
        dbg_hbn1 = nc.dram_tensor("dbg_hbn1", [P, NW, HC], F32, kind="ExternalOutput")
        dbg_pool = nc.dram_tensor("dbg_pool", [G, HC], F32, kind="ExternalOutput")

    # Internal DRAM
    tshardA = [nc.dram_tensor(f"tshardA{l}", [HALF_A, TWP], TBL_DT, kind="Internal") for l in (1, 2)]
    tshardB = [nc.dram_tensor(f"tshardB{l}", [HALF_B, TWP], TBL_DT, kind="Internal") for l in (1, 2)]
    tfullA = [
        nc.dram_tensor(f"tfullA{l}", [A_ROWS, TWP], TBL_DT, kind="Internal", addr_space="Shared")
        for l in (1, 2)
    ]
    tfullB = [
        nc.dram_tensor(f"tfullB{l}", [B_ROWS, TWP], TBL_DT, kind="Internal", addr_space="Shared")
        for l in (1, 2)
    ]
    stats_in = [nc.dram_tensor(f"statsin{l}", [1, 2 * HC], F32, kind="Internal") for l in (1, 2)]
    stats_out = [
        nc.dram_tensor(f"statsout{l}", [1, 2 * HC], F32, kind="Internal", addr_space="Shared")
        for l in (1, 2)
    ]
    pooled_in = nc.dram_tensor("pooledin", [G + 1, HC], F32, kind="Internal")
    pooled_out = nc.dram_tensor("pooledout", [G, HC], F32, kind="Internal", addr_space="Shared")

    from concourse.masks import make_identity

    with tile.TileContext(nc) as tc:
        with (
            tc.tile_pool(name="persist", bufs=1) as pp,
            tc.tile_pool(name="const", bufs=1) as cp,
            tc.tile_pool(name="work", bufs=2) as wp,
            tc.tile_pool(name="edge", bufs=2) as ep,
            tc.tile_pool(name="gath", bufs=2) as gp,
            tc.tile_pool(name="gbig", bufs=3) as gbp,
            tc.tile_pool(name="psum_tr", bufs=2, space="PSUM") as ptr,
            tc.tile_pool(name="psum_mm", bufs=2, space="PSUM") as pmm,
            tc.tile_pool(name="psum_agg", bufs=2, space="PSUM") as pagg,
            tc.tile_pool(name="psum_st", bufs=2, space="PSUM") as pst,
            # bank budget: tr 2 + psum_h 2 + agg 2 + st 2 = 8
        ):
            # ---------------- constants
            identity = cp.tile([P, P], F32)
            make_identity(nc, identity[:])
            identity_h = cp.tile([P, P], F16)
            nc.vector.tensor_copy(identity_h[:], identity[:])
            iota_i = cp.tile([P, P], I32)
            nc.gpsimd.iota(iota_i[:], pattern=[[1, P]], base=0, channel_multiplier=0)
            iota_f = cp.tile([P, P], F32)
            nc.vector.tensor_copy(iota_f[:], iota_i[:])
            iota_h = cp.tile([P, P], F16)
            nc.vector.tensor_copy(iota_h[:], iota_i[:])
            iota_ci = cp.tile([P, 1], I32)
            nc.gpsimd.iota(iota_ci[:], pattern=[[0, 1]], base=0, channel_multiplier=1)
            iota_col = cp.tile([P, 1], F32)
            nc.vector.tensor_copy(iota_col[:], iota_ci[:])
            ones_col = cp.tile([P, 1], F16)
            nc.gpsimd.memset(ones_col[:], 1.0)

            wcat1_s = cp.tile([D_IN, 2 * HC], F16)
            nc.sync.dma_start(wcat1_s[:], wcat1[:])
            wcat2_s0 = cp.tile([P, 2 * HC], F16)
            nc.sync.dma_start(wcat2_s0[:], wcat2[0:P, :])
            wcat2_s1 = cp.tile([P, 2 * HC], F16)
            nc.sync.dma_start(wcat2_s1[:], wcat2[P:HC, :])
            asrc_s = [cp.tile([P, HC], F32, name=f"asrc_s{l}") for l in range(2)]
            adst_s = [cp.tile([P, HC], F32, name=f"adst_s{l}") for l in range(2)]
            bias_s = [cp.tile([P, HC], F32, name=f"bias_s{l}") for l in range(2)]
            for l in range(2):
                nc.sync.dma_start(asrc_s[l][:], asrc[l][:])
                nc.sync.dma_start(adst_s[l][:], adst[l][:])
                nc.sync.dma_start(bias_s[l][:], biasr[l][:])

            # persistent node buffers
            h_pre = pp.tile([P, NW, HC], F16)   # pre/post BN node features
            ed_sb = pp.tile([P, NW, H], F32)    # per-node a_dst scores (this layer)
            nc.gpsimd.memset(h_pre[:], 0.0)

            def rows_of(w):
                return LAST_ROWS if w == NW - 1 else P

            # ================= per-layer =================
            for l in range(2):
                # ---- prelude: GEMM h||res, es/ed, build tables
                stats_acc = wp.tile([1, 2 * HC], F32, tag="stats_acc")
                nc.gpsimd.memset(stats_acc[:], 0.0)

                for w in range(NW):
                    rows = rows_of(w)
                    psum_h = pmm.tile([P, 2 * HC], F32, tag="psum_h")
                    if l == 0:
                        x_t = wp.tile([P, D_IN], F16, tag="x_t")
                        nc.sync.dma_start(x_t[:rows], xc[w * P : w * P + rows, :])
                        tr = ptr.tile([P, P], F16, tag="tr")
                        nc.tensor.transpose(tr[:, :rows], x_t[:rows, :], identity_h[:rows, :rows])
                        xT = wp.tile([P, P], F16, tag="xT")
                        nc.scalar.copy(xT[:, :rows], tr[:, :rows])
                        nc.tensor.matmul(
                            psum_h[:rows], lhsT=xT[:, :rows], rhs=wcat1_s[:], start=True, stop=True
                        )
                    else:
                        hin = h_pre[:, w, :]
                        for k in range(2):
                            tr = ptr.tile([P, P], F16, tag="tr")
                            nc.tensor.transpose(
                                tr[:, :rows], hin[:rows, k * P : (k + 1) * P], identity_h[:rows, :rows]
                            )
                            hT = wp.tile([P, P], F16, tag="xT")
                            nc.scalar.copy(hT[:, :rows], tr[:, :rows])
                            nc.tensor.matmul(
                                psum_h[:rows],
                                lhsT=hT[:, :rows],
                                rhs=(wcat2_s0 if k == 0 else wcat2_s1)[:],
                                start=(k == 0),
                                stop=(k == 1),
                            )
                    h_t = wp.tile([P, HC], F32, tag="h_t")
                    nc.scalar.copy(h_t[:rows], psum_h[:rows, 0:HC])
                    # residual + bias -> h_pre tile (overwrites layer-(l-1) values,
                    # which only this window's GEMM reads)
                    nc.vector.tensor_add(
                        h_pre[:rows, w, :], psum_h[:rows, HC : 2 * HC], bias_s[l][:rows]
                    )
                    # es / ed
                    scr = wp.tile([P, HC], F32, tag="scr")
                    es_t = wp.tile([P, H], F32, tag="es_t")
                    nc.vector.tensor_tensor(
                        out=scr[:rows], in0=h_t[:rows], in1=asrc_s[l][:rows], op=mybir.AluOpType.mult
                    )
                    nc.vector.reduce_sum(
                        out=es_t[:rows],
                        in_=scr[:rows].rearrange("p (h c) -> p h c", h=H),
                        axis=mybir.AxisListType.X,
                    )
                    nc.vector.tensor_tensor(
                        out=scr[:rows], in0=h_t[:rows], in1=adst_s[l][:rows], op=mybir.AluOpType.mult
                    )
                    nc.vector.reduce_sum(
                        out=ed_sb[:rows, w, :],
                        in_=scr[:rows].rearrange("p (h c) -> p h c", h=H),
                        axis=mybir.AxisListType.X,
                    )
                    # write table shard + ed table (fp16)
                    h_tb = wp.tile([P, HC], TBL_DT, tag="h_tb")
                    nc.scalar.copy(h_tb[:rows], h_t[:rows])
                    es_tb = wp.tile([P, H], TBL_DT, tag="es_tb")
                    nc.scalar.copy(es_tb[:rows], es_t[:rows])
                    if w < WA:
                        ro = w * P
                        nc.sync.dma_start(tshardA[l][ro : ro + rows, 0:HC], h_tb[:rows])
                        nc.sync.dma_start(tshardA[l][ro : ro + rows, HC:TW], es_tb[:rows])
                    else:
                        ro = (w - WA) * P
                        nc.sync.dma_start(tshardB[l][ro : ro + rows, 0:HC], h_tb[:rows])
                        nc.sync.dma_start(tshardB[l][ro : ro + rows, HC:TW], es_tb[:rows])
                    if w == WA - 1:
                        nc.gpsimd.collective_compute(
                            "AllGather",
                            mybir.AluOpType.bypass,
                            replica_groups=RG,
                            ins=[tshardA[l].ap()],
                            outs=[tfullA[l].ap()],
                        )

                # ---- all-gather table half B (half A launched mid-prelude)
                nc.gpsimd.collective_compute(
                    "AllGather",
                    mybir.AluOpType.bypass,
                    replica_groups=RG,
                    ins=[tshardB[l].ap()],
                    outs=[tfullB[l].ap()],
                )

                if dbg and l == 0:
                    nc.sync.dma_start(dbg_tfull[:], tfullA[0][:])

                # ---- edge pass; A-gathers prefetched PF windows ahead so they
                # run during AllGather-B and ahead of per-window compute
                PF = 1
                gtiles = {}

                def emit_gather_a(w):
                    li = gp.tile([P, NLOW * 8], I16, tag="li")
                    nc.sync.dma_start(li[:], lowidx_d[:, w * NLOW * 8 : (w + 1) * NLOW * 8])
                    g = gbp.tile([P, K_w, TWP], TBL_DT, tag="g")
                    nc.gpsimd.dma_gather(
                        out_ap=g[:, 0:NLOW, :],
                        in_ap=tfullA[l][:],
                        idxs_ap=li[:],
                        num_idxs=NLOW * P,
                        num_idxs_reg=NLOW * P,
                        elem_size=TWP,
                        single_packet=False,
                        queue_num=0,
                    )
                    gtiles[w] = g

                for w in range(PF):
                    emit_gather_a(w)

                for w in range(NW):
                    rows = rows_of(w)
                    g = gtiles.pop(w)
                    hi = gp.tile([P, NHIGH * 8], I16, tag="hi")
                    nc.sync.dma_start(hi[:], highidx_d[:, w * NHIGH * 8 : (w + 1) * NHIGH * 8])
                    drel = gp.tile([P, K_w], F16, tag="drel")
                    nc.sync.dma_start(drel[:], drel_d[:, w * K_w : (w + 1) * K_w])
                    drelT_rep = gp.tile([P, K_w * P], F16, tag="drelT_rep")
                    nc.sync.dma_start(
                        drelT_rep[:], drelT_d[w : w + 1, :].to_broadcast([P, K_w * P])
                    )
                    nc.gpsimd.dma_gather(
                        out_ap=g[:, NLOW:K_w, :],
                        in_ap=tfullB[l][:],
                        idxs_ap=hi[:],
                        num_idxs=NHIGH * P,
                        num_idxs_reg=NHIGH * P,
                        elem_size=TWP,
                        single_packet=False,
                        queue_num=1,
                    )
                    if w + PF < NW:
                        emit_gather_a(w + PF)
                    # ed broadcast node->edge via transposed indicator matmuls
                    ind_T = ep.tile([P, K_w, P], F16, tag="indT")
                    nc.vector.tensor_scalar(
                        out=ind_T[:],
                        in0=drelT_rep[:].rearrange("d (t e) -> d t e", e=P),
                        scalar1=iota_col[:, 0:1],
                        scalar2=None,
                        op0=mybir.AluOpType.is_equal,
                    )
                    ed16 = wp.tile([P, H], F16, tag="ed16")
                    nc.scalar.copy(ed16[:], ed_sb[:, w, :])
                    psum_ed = ptr.tile([P, K_w * H], F32, tag="tr")
                    for t in range(K_w):
                        nc.tensor.matmul(
                            psum_ed[:, t * H : (t + 1) * H],
                            lhsT=ind_T[:, t, :],
                            rhs=ed16[:],
                            start=True,
                            stop=True,
                        )
                    edv = wp.tile([P, K_w * H], F32, tag="edv")
                    nc.scalar.copy(edv[:], psum_ed[:])
                    # ex = exp(lrelu(es + ed))
                    s_t = wp.tile([P, K_w, H], F32, tag="s_t")
                    nc.vector.tensor_tensor(
                        out=s_t[:],
                        in0=g[:, :, HC:TW],
                        in1=edv[:].rearrange("p (t h) -> p t h", h=H),
                        op=mybir.AluOpType.add,
                    )
                    lr_t = wp.tile([P, K_w, H], F32, tag="lr_t")
                    nc.vector.tensor_scalar_mul(lr_t[:], s_t[:], SLOPE)
                    nc.vector.tensor_tensor(
                        out=lr_t[:], in0=lr_t[:], in1=s_t[:], op=mybir.AluOpType.max
                    )
                    ex_t = wp.tile([P, K_w, H], F16, tag="ex_t")
                    nc.scalar.activation(ex_t[:], lr_t[:], mybir.ActivationFunctionType.Exp)
                    # indicator
                    ind = ep.tile([P, K_w, P], IND_DT, tag="ind")
                    nc.vector.tensor_tensor(
                        out=ind[:],
                        in0=drel[:, :, None].to_broadcast([P, K_w, P]),
                        in1=iota_h[:, None, :].to_broadcast([P, K_w, P]),
                        op=mybir.AluOpType.is_equal,
                    )
                    # staging: [g*ex || ex]
                    stg = ep.tile([P, K_w, TW], STG_DT, tag="stg")
                    nc.vector.tensor_tensor(
                        out=stg[:, :, 0:HC].rearrange("p t (h c) -> p t h c", h=H),
                        in0=g[:, :, 0:HC].rearrange("p t (h c) -> p t h c", h=H),
                        in1=ex_t[:, :, :, None].to_broadcast([P, K_w, H, C]),
                        op=mybir.AluOpType.mult,
                    )
                    nc.scalar.copy(stg[:, :, HC:TW], ex_t[:])
                    if dbg and l == 0 and w == 0:
                        nc.gpsimd.dma_start(dbg_g[:], g[:])
                        nc.gpsimd.dma_start(dbg_stg[:], stg[:])
                        nc.gpsimd.dma_start(dbg_ind[:], ind[:])
                    # aggregate
                    agg = pagg.tile([P, TW], F32, tag="agg")
                    for t in range(K_w):
                        nc.tensor.matmul(
                            agg[:],
                            lhsT=ind[:, t, :],
                            rhs=stg[:, t, :],
                            start=(t == 0),
                            stop=(t == K_w - 1),
                        )
                    if dbg and l == 0 and w == 0:
                        aggc = wp.tile([P, TW], F32, tag="aggc")
                        nc.scalar.copy(aggc[:], agg[:])
                        nc.sync.dma_start(dbg_agg[:], aggc[:])
                    # evict: h_pre = agg[:, :256] * rden + res_b
                    den = wp.tile([P, H], F32, tag="den")
                    nc.vector.tensor_scalar_add(den[:rows], agg[:rows, HC:TW], 1e-16)
                    rden = wp.tile([P, H], F32, tag="rden")
                    nc.vector.reciprocal(rden[:rows], den[:rows])
                    agn = wp.tile([P, HC], F32, tag="agn")
                    nc.vector.tensor_tensor(
                        out=agn[:rows].rearrange("p (h c) -> p h c", h=H),
                        in0=agg[:rows, 0:HC].rearrange("p (h c) -> p h c", h=H),
                        in1=rden[:rows, :, None].to_broadcast([rows, H, C]),
                        op=mybir.AluOpType.mult,
                    )
                    nc.vector.tensor_add(h_pre[:rows, w, :], agn[:rows], h_pre[:rows, w, :])
                    # BN stats
                    hsq = wp.tile([P, HC], F16, tag="hsq")
                    nc.scalar.square(hsq[:rows], h_pre[:rows, w, :])
                    st = pst.tile([1, 2 * HC], F32, tag="st")
                    nc.tensor.matmul(
                        st[0:1, 0:HC], lhsT=ones_col[:rows, :], rhs=h_pre[:rows, w, :], start=True, stop=True
                    )
                    nc.tensor.matmul(
                        st[0:1, HC : 2 * HC], lhsT=ones_col[:rows, :], rhs=hsq[:rows], start=True, stop=True
                    )
                    nc.vector.tensor_add(stats_acc[:], stats_acc[:], st[:])

                if dbg and l == 0:
                    nc.gpsimd.dma_start(dbg_hpre1[:], h_pre[:])
                # ---- BN all-reduce + apply + relu
                nc.sync.dma_start(stats_in[l][:], stats_acc[:])
                nc.gpsimd.collective_compute(
                    "AllReduce",
                    mybir.AluOpType.add,
                    replica_groups=RG,
                    ins=[stats_in[l].ap()],
                    outs=[stats_out[l].ap()],
                )
                stats_g = wp.tile([1, 2 * HC], F32, tag="stats_g")
                nc.sync.dma_start(stats_g[:], stats_out[l][:])
                bngs = wp.tile([1, HC], F32, tag="bngs")
                nc.sync.dma_start(bngs[:], bng[l][:])
                bnbs = wp.tile([1, HC], F32, tag="bnbs")
                nc.sync.dma_start(bnbs[:], bnb[l][:])
                mrow = wp.tile([1, HC], F32, tag="mrow")
                nc.vector.tensor_scalar_mul(mrow[:], stats_g[0:1, 0:HC], 1.0 / N)
                vrow = wp.tile([1, HC], F32, tag="vrow")
                nc.vector.tensor_scalar_mul(vrow[:], stats_g[0:1, HC : 2 * HC], 1.0 / N)
                msq = wp.tile([1, HC], F32, tag="msq")
                nc.vector.tensor_tensor(out=msq[:], in0=mrow[:], in1=mrow[:], op=mybir.AluOpType.mult)
                nc.vector.tensor_tensor(out=vrow[:], in0=vrow[:], in1=msq[:], op=mybir.AluOpType.subtract)
                nc.vector.tensor_scalar_add(vrow[:], vrow[:], EPS)
                sq = wp.tile([1, HC], F32, tag="sq")
                nc.scalar.activation(sq[:], vrow[:], mybir.ActivationFunctionType.Sqrt)
                inv = wp.tile([1, HC], F32, tag="inv")
                nc.vector.reciprocal(inv[:], sq[:])
                scale = wp.tile([1, HC], F32, tag="scale")
                nc.vector.tensor_tensor(out=scale[:], in0=inv[:], in1=bngs[:], op=mybir.AluOpType.mult)
                shift = wp.tile([1, HC], F32, tag="shift")
                nc.vector.tensor_tensor(out=shift[:], in0=mrow[:], in1=scale[:], op=mybir.AluOpType.mult)
                nc.vector.tensor_tensor(out=shift[:], in0=bnbs[:], in1=shift[:], op=mybir.AluOpType.subtract)
                scale_rep = wp.tile([P, HC], F32, tag="scale_rep")
                nc.gpsimd.partition_broadcast(scale_rep[:], scale[:])
                shift_rep = wp.tile([P, HC], F32, tag="shift_rep")
                nc.gpsimd.partition_broadcast(shift_rep[:], shift[:])
                for w in range(NW):
                    rows = rows_of(w)
                    nc.vector.tensor_tensor(
                        out=h_pre[:rows, w, :], in0=h_pre[:rows, w, :], in1=scale_rep[:rows],
                        op=mybir.AluOpType.mult,
                    )
                    nc.vector.tensor_tensor(
                        out=h_pre[:rows, w, :], in0=h_pre[:rows, w, :], in1=shift_rep[:rows],
                        op=mybir.AluOpType.add,
                    )
                    nc.scalar.activation(
                        h_pre[:rows, w, :], h_pre[:rows, w, :], mybir.ActivationFunctionType.Relu
                    )
                if dbg and l == 0:
                    nc.sync.dma_start(dbg_stats[:], stats_out[0][:])
                    nc.gpsimd.dma_start(dbg_hbn1[:], h_pre[:])

            # ================= pooling =================
            pool_ps = pmm.tile([P, HC], F32, tag="psum_h")
            for w in range(NW):
                rows = rows_of(w)
                grt = wp.tile([P, 1], F32, tag="grt")
                nc.sync.dma_start(grt[:], grel_d[:, w : w + 1])
                gind = wp.tile([P, P], F16, tag="gind")
                nc.vector.tensor_tensor(
                    out=gind[:],
                    in0=grt[:].to_broadcast([P, P]),
                    in1=iota_f[:],
                    op=mybir.AluOpType.is_equal,
                )
                nc.tensor.matmul(
                    pool_ps[:],
                    lhsT=gind[:rows, :],
                    rhs=h_pre[:rows, w, :],
                    start=(w == 0),
                    stop=(w == NW - 1),
                )
            pooled_sb = wp.tile([P, HC], F32, tag="pooled_sb")
            nc.scalar.copy(pooled_sb[:], pool_ps[:])
            zt = wp.tile([P, HC], F32, tag="zt")
            nc.gpsimd.memset(zt[:], 0.0)
            for r in range(0, G, P):
                nc.sync.dma_start(pooled_in[r : r + P, :], zt[:])
            nc.sync.dma_start(pooled_in[G : G + 1, :], zt[0:1, :])
            scat = wp.tile([P, 1], I32, tag="scat")
            nc.sync.dma_start(scat[:], scatidx_d[:])
            nc.gpsimd.indirect_dma_start(
                out=pooled_in[:],
                out_offset=bass.IndirectOffsetOnAxis(ap=scat[:, :1], axis=0),
                in_=pooled_sb[:],
                in_offset=None,
            )
            nc.gpsimd.collective_compute(
                "AllReduce",
                mybir.AluOpType.add,
                replica_groups=RG,
                ins=[pooled_in[0:G, :]],
                outs=[pooled_out.ap()],
            )

            if dbg:
                nc.sync.dma_start(dbg_pool[:], pooled_out[:])

            # ================= MLP =================
            fc1w_s0 = cp.tile([P, 128], F32)
            nc.sync.dma_start(fc1w_s0[:], fc1w[0:P, :])
            fc1w_s1 = cp.tile([P, 128], F32)
            nc.sync.dma_start(fc1w_s1[:], fc1w[P:HC, :])
            fc2w_s = cp.tile([P, 2], F32)
            nc.sync.dma_start(fc2w_s[:], fc2w[:])
            fc1b_s = cp.tile([P, 128], F32)
            nc.sync.dma_start(fc1b_s[:], fc1b[:])
            fc2b_s = cp.tile([P, 2], F32)
            nc.sync.dma_start(fc2b_s[:], fc2b[:])

            for tgi in range(G // P):
                pt = wp.tile([P, HC], F32, tag="pt")
                nc.sync.dma_start(pt[:], pooled_out[tgi * P : (tgi + 1) * P, :])
                rc = wp.tile([P, 1], F32, tag="rc")
                nc.sync.dma_start(rc[:], rcnt[tgi * P : (tgi + 1) * P, :])
                nc.vector.tensor_scalar_mul(pt[:], pt[:], rc[:, 0:1])
                mm1 = pmm.tile([P, 128], F32, tag="psum_h")
                for k in range(2):
                    tr = ptr.tile([P, P], F32, tag="tr")
                    nc.tensor.transpose(tr[:], pt[:, k * P : (k + 1) * P], identity[:])
                    ptT = wp.tile([P, P], F32, tag="ptT")
                    nc.scalar.copy(ptT[:], tr[:])
                    nc.tensor.matmul(
                        mm1[:],
                        lhsT=ptT[:],
                        rhs=(fc1w_s0 if k == 0 else fc1w_s1)[:],
                        start=(k == 0),
                        stop=(k == 1),
                    )
                o1 = wp.tile([P, 128], F32, tag="o1")
                nc.vector.tensor_add(o1[:], mm1[:], fc1b_s[:])
                nc.scalar.activation(o1[:], o1[:], mybir.ActivationFunctionType.Relu)
                tr2 = ptr.tile([P, P], F32, tag="tr")
                nc.tensor.transpose(tr2[:], o1[:], identity[:])
                o1T = wp.tile([P, P], F32, tag="o1T")
                nc.scalar.copy(o1T[:], tr2[:])
                mm2 = pagg.tile([P, 2], F32, tag="agg")
                nc.tensor.matmul(mm2[:], lhsT=o1T[:], rhs=fc2w_s[:], start=True, stop=True)
                oo = wp.tile([P, 2], F32, tag="oo")
                nc.vector.tensor_add(oo[:], mm2[:], fc2b_s[:])
                nc.sync.dma_start(out_d[tgi * P : (tgi + 1) * P, :], oo[:])

    nc.compile()
    return nc


_PROGRAM_CACHE: dict[tuple, object] = {}


def _get_program(split: tuple):
    if split not in _PROGRAM_CACHE:
        _PROGRAM_CACHE[split] = _build_program(split)
    return _PROGRAM_CACHE[split]


# ---------------------------------------------------------------- entry point

def kernel(
    x, edge_index, edge_attr, batch,
    W1, att_src1, att_dst1, b1, res1_W, res1_b, bn1_g, bn1_b,
    W2, att_src2, att_dst2, b2, res2_W, res2_b, bn2_g, bn2_b,
    fc1_W, fc1_b, fc2_W, fc2_b,
    _run_opts: dict | None = None,
):
    x = np.asarray(x, np.float32)
    edge_index = np.asarray(edge_index)
    batch = np.asarray(batch)

    metas, split = _host_meta(edge_index, batch)
    nc = _get_program(split)

    wcat1 = np.ascontiguousarray(np.concatenate([np.asarray(W1), np.asarray(res1_W)], axis=1)).astype(np.float16)
    wcat2 = np.ascontiguousarray(np.concatenate([np.asarray(W2), np.asarray(res2_W)], axis=1)).astype(np.float16)
    rep = lambda v: np.ascontiguousarray(np.tile(np.asarray(v, np.float32).reshape(1, -1), (P, 1)))
    cnt = np.bincount(batch, minlength=G).astype(np.float32)
    rcnt = (1.0 / np.maximum(cnt, 1.0)).astype(np.float32)[:, None]

    shared = dict(
        wcat1=wcat1, wcat2=wcat2,
        asrc1=rep(np.asarray(att_src1).reshape(-1)), adst1=rep(np.asarray(att_dst1).reshape(-1)),
        asrc2=rep(np.asarray(att_src2).reshape(-1)), adst2=rep(np.asarray(att_dst2).reshape(-1)),
        bias1=rep(np.asarray(b1) + np.asarray(res1_b)), bias2=rep(np.asarray(b2) + np.asarray(res2_b)),
        bng1=np.asarray(bn1_g, np.float32).reshape(1, -1), bnb1=np.asarray(bn1_b, np.float32).reshape(1, -1),
        bng2=np.asarray(bn2_g, np.float32).reshape(1, -1), bnb2=np.asarray(bn2_b, np.float32).reshape(1, -1),
        fc1w=np.asarray(fc1_W, np.float32), fc2w=np.asarray(fc2_W, np.float32),
        fc1b=rep(fc1_b), fc2b=rep(fc2_b),
        rcnt=rcnt,
    )
    in_maps = []
    for c in range(M):
        m = metas[c]
        in_maps.append(
            dict(
                shared,
                xc=np.ascontiguousarray(x[c * NCN : (c + 1) * NCN]).astype(np.float16),
                grel=m["grel"], scatidx=m["scatidx"],
                lowidx=m["lowidx"], highidx=m["highidx"], drel=m["drel"], drelT=m["drelT"],
            )
        )

    opts = _run_opts or {}
    res = run_bass_kernel_spmd(nc, in_maps, core_ids=list(range(M)), **opts)
    out = res.results[0]["out"].astype(np.float32)
    if opts:
        kernel.last_result = res  # stash for profiling harnesses
    return out
